# revision 25
# baseline (speedup 1.0000x reference)
"""Distributed single-head attention kernel for one TRN2 chip (8 NeuronCores).

Problem: x[8192,1024] fp32; q/k/v = x@W* + b*; out = softmax(q k^T / 8) @ v.

Strategy (sequence parallel, fully collective-free):
  - shard the QUERY rows of x across 8 cores; REPLICATE x (bf16, 16MB)
    to every core so each computes all of k and v locally. Measured on
    this part, the first collective cannot start before ~65-75us (CC
    runtime rendezvous gated by cross-core start skew, independent of
    trigger time), which idled the PE for ~30us; recomputing k/v for
    the 7 remote shards costs ~35us of PE work that replaces that idle
    and removes all CC latency/variance and the end-of-kernel CC sync
  - HOST pre-packs the inputs: x cast to bf16 and pre-transposed into
    the exact [partition, shard, chunk, m] SBUF layout (16KB DMA lines),
    ROTATED per core so shard slot 0 is always the core's own rows (no
    device-side rank logic); weights pre-cast to bf16 with Wk|Wv packed
    into one [128, 128] lhsT so k and v project in a single matmul chain
  - shard tiles stream in round-robin over the three DMA queues
    (sync/scalar/gpsimd) and recycle through a 4-deep pool
  - attention is computed transposed: S^T[n,m] = K @ q^T so softmax's
    n-dimension lands on partitions; the row-sum comes free from a ones
    column appended to V (V_aug): out^T = V_aug^T @ E^T accumulates
    numerator and denominator in one PSUM chain
  - the packed kvT tile is used directly as the S-matmul lhsT with the
    contraction padded to K=128 (qT's bottom 64 partitions are zeroed,
    nulling the v-junk rows): the HAM clock gate does not count K=64
    matmuls as PE-busy and would hold the PE at 1.2 GHz for the whole
    attention loop; K=128 padding keeps it at 2.4 GHz at zero cycle cost
  - exp alternates between ScalarE (native) and VectorE (Schraudolph
    bit-trick emitting the bf16 pattern via an int16 convert)
  - finalize: transpose out^T back (bf16), normalize by reciprocal
    row-sum, +bv

Math shortcuts (exactness preserved):
  - softmax(s + c_row) == softmax(s): the k-bias term is row-constant -> bk
    dropped entirely
  - softmax rows sum to 1 -> v-bias added after the weighted sum
  - logits are ~N(0,1), exp cannot overflow in fp32 -> no max pass
  - k/v recomputed locally are bitwise identical to what a gather would
    deliver (same bf16 inputs, same matmul), so accuracy is unchanged
"""

import sys

if "/opt/trn_rl_repo" not in sys.path:
    sys.path.insert(0, "/opt/trn_rl_repo")

import math

import numpy as np

N, D, H = 8192, 1024, 64
NCORES = 8
ML = N // NCORES          # rows per core: 1024
P = 128
CCH = D // P              # contraction chunks over D: 8
MT = ML // P              # 128-row tiles per shard: 8
NCH = N // P              # total key chunks of 128: 64
SCALE = float(H) ** -0.5
PIPE_D = 4                # V-matmul runs this many chunks behind the S/exp
SH_COLS = CCH * ML        # flattened xT columns per shard (8192)

# Schraudolph exp producing a bf16 bit pattern in int16:
#   bf16_bits(exp(scale*s)) ~= round(A16*s + B16)
A16 = SCALE * math.log2(math.e) * 2.0**7
B16 = 127.0 * 2.0**7 - 0.06 * 2.0**7   # c=0.06 tuned for end-to-end error

_CACHE = {}


def _build():
    from concourse import bacc, mybir, tile, masks

    F32 = mybir.dt.float32
    BF16 = mybir.dt.bfloat16
    I16 = mybir.dt.int16
    AF = mybir.ActivationFunctionType
    ADD = mybir.AluOpType.add
    MULT = mybir.AluOpType.mult

    nc = bacc.Bacc("TRN2", target_bir_lowering=False, debug=False,
                   num_devices=NCORES)

    xt_d = nc.dram_tensor("xt", [P, NCORES * SH_COLS], BF16,
                          kind="ExternalInput")
    wkv_d = nc.dram_tensor("wkv", [P, CCH * P], BF16, kind="ExternalInput")
    wq_d = nc.dram_tensor("wq", [P, CCH * H], BF16, kind="ExternalInput")
    bq_d = nc.dram_tensor("bq", [H, 1], F32, kind="ExternalInput")
    bv_d = nc.dram_tensor("bv", [1, H], F32, kind="ExternalInput")
    out_d = nc.dram_tensor("out", [ML, H], F32, kind="ExternalOutput")

    with tile.TileContext(nc) as tc:
        with (
            tc.tile_pool(name="constp", bufs=1) as constp,
            tc.tile_pool(name="wtsp", bufs=1) as wtsp,
            tc.tile_pool(name="xinp", bufs=4) as xinp,
            tc.tile_pool(name="qkvp", bufs=1) as qkvp,
            tc.tile_pool(name="eTp", bufs=16) as eTp,
            tc.tile_pool(name="finp", bufs=2) as finp,
        ):
            # ---- weight + bias loads (small, gpsimd queue first) ----
            wkv_sb = wtsp.tile([P, CCH * P], BF16, tag="wkv")
            nc.gpsimd.dma_start(wkv_sb[:], wkv_d[:, :])
            wq_sb = wtsp.tile([P, CCH * H], BF16, tag="wq")
            nc.gpsimd.dma_start(wq_sb[:], wq_d[:, :])
            bq_sb = constp.tile([H, 1], F32, tag="bq")
            nc.gpsimd.dma_start(bq_sb[:], bq_d[:, :])
            bv_sb = constp.tile([1, H], F32, tag="bv")
            nc.gpsimd.dma_start(bv_sb[:], bv_d[:, :])

            # ---- x shard loads: round-robin over the 3 DMA queues.
            # Shard 0 (own rows, needed first for q + first chunks) is
            # split across sync+scalar so it lands in ~8us.
            xh = []
            for j in range(NCORES):
                xj = xinp.tile([P, SH_COLS], BF16, tag="xh", name=f"xh_{j}")
                src = xt_d[:, SH_COLS * j:SH_COLS * (j + 1)]
                if j == 0:
                    nc.sync.dma_start(xj[0:64, :], src[0:64, :])
                    nc.scalar.dma_start(xj[64:P, :], src[64:P, :])
                else:
                    eng = (nc.gpsimd, nc.sync, nc.scalar)[j % 3]
                    eng.dma_start(xj[:], src)
                xh.append(xj)

            # ---- constants ----
            id_bf = constp.tile([P, P], BF16, tag="id_bf")
            masks.make_identity(nc, id_bf[:])
            ones1 = constp.tile([1, P], F32, tag="ones1")
            nc.vector.memset(ones1[:], 1.0)
            bvb = constp.tile([P, H], F32, tag="bvb")  # bv broadcast to rows

            # packed kvT for all shards: rows 0:64 = kT, 64:128 = vT;
            # used directly as the (K=128-padded) S-matmul lhsT
            kvT_all = qkvp.tile([P, NCORES, ML], BF16, tag="kvT")
            # v natural [key, h|1] for all shards (ones col for row-sums)
            v_all = qkvp.tile([P, NCH, H + 1], BF16, tag="v_nat")
            nc.vector.memset(v_all[:, :, H:H + 1], 1.0)
            # qT padded to 128 partitions with a zero bottom half (see
            # module docstring: K=128 keeps the HAM clock gate warm)
            qT_sb = qkvp.tile([P, ML], BF16, tag="qT")
            nc.vector.memset(qT_sb[H:P, :], 0.0)

            # PSUM budget is exactly 8 banks: sT 2 bufs x 2 banks, oT 2
            # banks, and one shared 2-buf pool (1 bank each) that the
            # kv/q accumulator chains, v transposes and bvb rotate through
            with (
                tc.tile_pool(name="ps_a", bufs=2, space="PSUM") as ps_a,
                tc.tile_pool(name="ps_sT", bufs=2, space="PSUM") as ps_sT,
                tc.tile_pool(name="ps_oT", bufs=1, space="PSUM") as ps_oT,
            ):
                def kv_proj(j):
                    # packed [Wk|Wv] lhsT: k and v in one matmul chain
                    for h2 in range(2):
                        acc = ps_a.tile([P, 512], F32, tag="acc",
                                        name=f"acc_kv_{j}_{h2}")
                        for ch in range(CCH):
                            nc.tensor.matmul(
                                acc[:], wkv_sb[:, P * ch:P * (ch + 1)],
                                xh[j][:, ML * ch + 512 * h2:
                                      ML * ch + 512 * (h2 + 1)],
                                start=(ch == 0), stop=(ch == CCH - 1))
                        eng = nc.scalar if (j + h2) % 2 == 0 else nc.vector
                        (eng.copy if eng is nc.scalar
                         else eng.tensor_copy)(
                            kvT_all[:, j, 512 * h2:512 * (h2 + 1)], acc[:])
                    # v natural tiles via PE transpose (identity block at
                    # partitions 64:128 matches the v rows' base partition)
                    for t in range(MT):
                        vps = ps_a.tile([P, H], BF16, tag="acc",
                                        name=f"vps_{j}_{t}")
                        nc.tensor.transpose(
                            vps[:], kvT_all[H:P, j, P * t:P * (t + 1)],
                            id_bf[H:P, H:P])
                        eng = nc.scalar if t % 2 == 0 else nc.vector
                        (eng.copy if eng is nc.scalar
                         else eng.tensor_copy)(
                            v_all[:, MT * j + t, 0:H], vps[:])

                # q projection from shard 0 (the core's own rows)
                kv_proj(0)
                for h2 in range(2):
                    acc = ps_a.tile([H, 512], F32, tag="acc",
                                    name=f"acc_q_{h2}")
                    for ch in range(CCH):
                        nc.tensor.matmul(
                            acc[:], wq_sb[:, H * ch:H * (ch + 1)],
                            xh[0][:, ML * ch + 512 * h2:
                                  ML * ch + 512 * (h2 + 1)],
                            start=(ch == 0), stop=(ch == CCH - 1))
                    nc.vector.tensor_scalar_add(qT_sb[0:H, 512 * h2:
                                                      512 * (h2 + 1)],
                                                acc[:], bq_sb[:])

                # bv broadcast via rank-1 matmul: ones[1,128]^T @ bv[1,64]
                bvb_ps = ps_a.tile([P, H], F32, tag="acc")
                nc.tensor.matmul(bvb_ps[:], ones1[:], bv_sb[:],
                                 start=True, stop=True)
                nc.vector.tensor_copy(bvb[:], bvb_ps[:])

                # ---- attention: S^T = K qT; E^T = exp(S^T/8);
                #      O^T += Vaug^T E^T, pipelined PIPE_D chunks behind
                oT = ps_oT.tile([H + 1, ML], F32, tag="oT")
                eTs = []

                def chunk(i):
                    j, c = divmod(i, MT)
                    sT = ps_sT.tile([P, ML], F32, tag="sT", name=f"sT_{i}")
                    for h2 in range(2):
                        msl = slice(512 * h2, 512 * (h2 + 1))
                        nc.tensor.matmul(
                            sT[:, msl], kvT_all[:, j, P * c:P * (c + 1)],
                            qT_sb[:, msl], start=True, stop=True)
                    if i % 2 == 0:
                        eT = eTp.tile([P, ML], BF16, tag="eT", name=f"eT_{i}")
                        nc.scalar.activation(eT[:], sT[:], AF.Exp, scale=SCALE)
                        eTs.append(eT)
                    else:
                        eTi = eTp.tile([P, ML], I16, tag="eT", name=f"eTi_{i}")
                        nc.vector.tensor_scalar(eTi[:], sT[:], A16, B16,
                                                op0=MULT, op1=ADD)
                        eTs.append(eTi.bitcast(BF16))
                    if i >= PIPE_D:
                        _accum_v(nc, oT, v_all, eTs[i - PIPE_D], i - PIPE_D)

                # per shard: kv projection, then its 8 chunks — keeps the
                # PE stream dense while later shards' DMAs are in flight
                for i in range(MT):
                    chunk(i)
                for j in range(1, NCORES):
                    kv_proj(j)
                    for i in range(MT * j, MT * (j + 1)):
                        chunk(i)
                for i in range(NCH - PIPE_D, NCH):
                    _accum_v(nc, oT, v_all, eTs[i], i)

                # ---- finalize: transpose back (bf16), normalize, +bv ----
                oT_sb = qkvp.tile([H + 1, ML], BF16, tag="oT_sb")
                for t in range(MT):
                    eng = nc.scalar if t % 2 == 0 else nc.vector
                    (eng.copy if eng is nc.scalar else eng.tensor_copy)(
                        oT_sb[:, P * t:P * (t + 1)], oT[:, P * t:P * (t + 1)])
                for t in range(MT):
                    ft = ps_sT.tile([P, H + 1], BF16, tag="sT",
                                    name=f"ft_{t}")
                    nc.tensor.transpose(
                        ft[:], oT_sb[:, P * t:P * (t + 1)],
                        id_bf[:H + 1, :H + 1])
                    rcp = finp.tile([P, 1], F32, tag="rcp", name=f"rcp_{t}")
                    nc.vector.reciprocal(rcp[:], ft[:, H:H + 1])
                    res = finp.tile([P, H], F32, tag="res", name=f"res_{t}")
                    # fused (numerator * 1/rowsum) + bv in one DVE op
                    nc.vector.scalar_tensor_tensor(
                        res[:], ft[:, 0:H], rcp[:], bvb[:],
                        op0=MULT, op1=ADD)
                    eng = nc.sync if t % 2 == 0 else nc.scalar
                    eng.dma_start(out_d[P * t:P * (t + 1), :], res[:])

    nc.compile()
    return nc


def _accum_v(nc, oT, v_all, eT, i):
    for h2 in range(2):
        msl = slice(512 * h2, 512 * (h2 + 1))
        nc.tensor.matmul(oT[:, msl], v_all[:, i, :], eT[:, msl],
                         start=(i == 0), stop=(i == NCH - 1),
                         skip_group_check=True)


def _get_nc():
    if "nc" not in _CACHE:
        _CACHE["nc"] = _build()
    return _CACHE["nc"]


def _prep_inputs(inputs):
    import ml_dtypes

    bf16 = ml_dtypes.bfloat16
    wkv = np.concatenate(
        [np.asarray(inputs["Wk"], dtype=np.float32),
         np.asarray(inputs["Wv"], dtype=np.float32)], axis=1).astype(bf16)
    wkv_p = np.ascontiguousarray(
        wkv.reshape(CCH, P, P).transpose(1, 0, 2).reshape(P, CCH * P))
    wq = np.asarray(inputs["Wq"], dtype=np.float32).astype(bf16)
    wq_p = np.ascontiguousarray(
        wq.reshape(CCH, P, H).transpose(1, 0, 2).reshape(P, CCH * H))
    bq = np.ascontiguousarray(
        inputs["bq"], dtype=np.float32).reshape(H, 1)
    bv = np.ascontiguousarray(
        inputs["bv"], dtype=np.float32).reshape(1, H)

    x = np.asarray(inputs["x"], dtype=np.float32)
    # per-shard packed layout [p, c*1024+m] with 16KB-contiguous lines
    shards = []
    for j in range(NCORES):
        xs = x[ML * j:ML * (j + 1)].astype(bf16)      # [m, d]
        t = xs.T.reshape(CCH, P, ML)                  # [c, p, m]
        shards.append(np.ascontiguousarray(
            t.transpose(1, 0, 2).reshape(P, SH_COLS)))
    in_maps = []
    for i in range(NCORES):
        # rotate so shard slot 0 is core i's own rows — the kernel then
        # needs no device-side rank logic; key order differs per core but
        # softmax sums over all keys, so the result is unchanged
        xt = np.concatenate([shards[(i + j) % NCORES]
                             for j in range(NCORES)], axis=1)
        in_maps.append({
            "xt": np.ascontiguousarray(xt), "wkv": wkv_p, "wq": wq_p,
            "bq": bq, "bv": bv,
        })
    return in_maps


def _run(inputs, trace=False, **kw):
    from concourse.bass_utils import run_bass_kernel_spmd

    nc = _get_nc()
    in_maps = _prep_inputs(inputs)
    res = run_bass_kernel_spmd(nc, in_maps, core_ids=list(range(NCORES)),
                               trace=trace, **kw)
    out = np.concatenate([res.results[i]["out"] for i in range(NCORES)],
                         axis=0)
    return out, res


def kernel(x, Wq, bq, Wk, bk, Wv, bv):
    out, _ = _run({"x": x, "Wq": Wq, "bq": bq, "Wk": Wk, "Wv": Wv, "bv": bv})
    return out
